# revision 8
# baseline (speedup 1.0000x reference)
"""Causal attention head (B=4, S=4096, D=512, E=64) on 8 TRN2 NeuronCores.

Sharding: per batch b, core pair (2b, 2b+1); each core owns 2048 queries
(zig-zag slots) and projects K/V for the full sequence.

v3 structure:
 - Per-512-seq-group input tiles, DMA issued in consumption order; compute
   chases the DMA stream (projections + attention interleaved in program
   order).
 - K and V^T projections as col-tiled concurrent matmul pairs
   (wk -> psum[0:64], wv -> psum[64:128]); one DVE copy moves both to SBUF
   (kv2 tiles hold 2 seq groups each).
 - Scores are col-tiled concurrent pairs over key-halves of one chunk
   (keys 0:64 -> psum[0:64], keys 64:128 -> psum[64:128]); both tiles share
   the same K^T stationary rows and Q^T moving operand from partitions 0:64,
   so no partition duplication is needed anywhere.
 - V^T -> V[keys, e] via 32x32 block-swap SBUF DMAs (gpsimd queue, batched
   per 2 groups) + DVE StreamTranspose.
 - PE warm-up matmuls + exp table-load primer at t=0.
 - Flash-style attention: exp on ScalarE over [128, 1024] chunk pairs, PV
   with a ones-column appended to V so the softmax denominator falls out of
   the same matmul. Uniform per-slot key-chunk counts {8,16,24,32}; zig-zag
   query slots; diagonal/zero masks multiply exp output.
All matmul inputs bf16 (pre-cast on host). Output f32.
"""

import sys

sys.path.insert(0, "/opt/trn_rl_repo")

import numpy as np
import ml_dtypes

from concourse import bacc, mybir
from concourse import tile
from concourse.bass_utils import run_bass_kernel_spmd

BF16 = ml_dtypes.bfloat16
F32 = mybir.dt.float32
BF = mybir.dt.bfloat16

B, S, D, E = 4, 4096, 512, 64
P = 128
NQ = 2048          # queries per core
QBLK = 512         # query block
NCH = D // P       # 4 contraction chunks for projections
NG = S // QBLK     # 8 seq groups of 512
NQG = NQ // QBLK   # 4 query groups
QSTARTS = {0: [0, 1024, 2048, 3072], 1: [512, 1536, 2560, 3584]}
SLOT_J = [8, 16, 24, 32]  # uniform per-slot key-chunk counts (all cores)

_CACHE = {}
LAST_RESULT = None


def _build():
    nc = bacc.Bacc(
        "TRN2",
        target_bir_lowering=False,
        debug=False,
        enable_asserts=True,
        num_devices=8,
    )

    xqt_d = nc.declare_dram_parameter("xqt", [D, NQ], BF, isOutput=False)
    xkt_d = nc.declare_dram_parameter("xkt", [D, S], BF, isOutput=False)
    xvt_d = nc.declare_dram_parameter("xvt", [D, S], BF, isOutput=False)
    wq = nc.declare_dram_parameter("wq", [D, E], BF, isOutput=False)  # pre-scaled 1/8
    wk = nc.declare_dram_parameter("wk", [D, E], BF, isOutput=False)
    wv = nc.declare_dram_parameter("wv", [D, E], BF, isOutput=False)
    masks = nc.declare_dram_parameter("masks", [P, 8 * QBLK], BF, isOutput=False)
    ident = nc.declare_dram_parameter("ident", [P, P], F32, isOutput=False)
    zout = nc.declare_dram_parameter("z", [NQ, E], F32, isOutput=True)

    with tile.TileContext(nc) as tc:
        with (
            tc.tile_pool(name="const", bufs=1) as const,
            tc.tile_pool(name="xin", bufs=1) as xin,
            tc.tile_pool(name="proj", bufs=1) as proj,
            tc.tile_pool(name="work", bufs=4) as work,
            tc.tile_pool(name="epi", bufs=2) as epi,
            tc.tile_pool(name="psP", bufs=2, space="PSUM") as psP,
            tc.tile_pool(name="psS", bufs=2, space="PSUM") as psS,
            tc.tile_pool(name="psZ", bufs=2, space="PSUM") as psZ,
        ):
            # ---- constants (DMA first: tiny) ----
            ident_sb = const.tile([P, P], F32, tag="ident")
            nc.sync.dma_start(out=ident_sb[:, :], in_=ident[:, :])
            wq_sb = const.tile([P, NCH, E], BF, tag="wq")
            wk_sb = const.tile([P, NCH, E], BF, tag="wk")
            wv_sb = const.tile([P, NCH, E], BF, tag="wv")
            for w_dram, w_sb in ((wk, wk_sb), (wv, wv_sb), (wq, wq_sb)):
                nc.sync.dma_start(
                    out=w_sb[:, :, :],
                    in_=w_dram.rearrange("(c p) e -> p c e", p=P),
                )

            # ---- input tiles (per 512-seq group, no rotation) ----
            xk_t = [
                xin.tile([P, NCH, QBLK], BF, tag=f"xk{g}", name=f"xk_t{g}")
                for g in range(NG)
            ]
            xv_t = [
                xin.tile([P, NCH, QBLK], BF, tag=f"xv{g}", name=f"xv_t{g}")
                for g in range(NG)
            ]
            xq_t = [
                xin.tile([P, NCH, QBLK], BF, tag=f"xq{b}", name=f"xq_t{b}")
                for b in range(NQG)
            ]
            masks_sb = const.tile([P, 8 * QBLK], BF, tag="masks")

            def load_seg(dram, t, g):
                nc.sync.dma_start(
                    out=t[:, :, :],
                    in_=dram[:, g * QBLK : (g + 1) * QBLK].rearrange(
                        "(c p) r -> p c r", p=P
                    ),
                )

            # DMA issue order == desired arrival order (sync HW queue).
            load_seg(xkt_d, xk_t[0], 0)
            load_seg(xvt_d, xv_t[0], 0)
            load_seg(xqt_d, xq_t[0], 0)
            load_seg(xkt_d, xk_t[1], 1)
            load_seg(xvt_d, xv_t[1], 1)
            nc.sync.dma_start(out=masks_sb[:, :], in_=masks[:, :])
            load_seg(xqt_d, xq_t[1], 1)
            for g in range(2, NG):
                load_seg(xkt_d, xk_t[g], g)
                load_seg(xvt_d, xv_t[g], g)
                if g < NQG:
                    load_seg(xqt_d, xq_t[g], g)

            # ---- projected-data tiles ----
            # kv2_g[gg] holds 2 seq groups: K^T in rows 0:64, V^T in 64:128
            kv2_g = [
                proj.tile([P, 2 * QBLK], BF, tag=f"kv{gg}", name=f"kv2_g{gg}")
                for gg in range(NG // 2)
            ]
            qt_b = [
                proj.tile([E, QBLK], BF, tag=f"qt{b}", name=f"qt_b{b}")
                for b in range(NQG)
            ]
            # vp2_g[gg]: V[keys, e] (+ones col) for 8 chunks (2 seq groups)
            vp2_g = [
                proj.tile([P, 2 * NCH, E + 1], BF, tag=f"vp{gg}", name=f"vp2_g{gg}")
                for gg in range(NG // 2)
            ]
            for gg in range(NG // 2):
                nc.gpsimd.memset(vp2_g[gg][:, :, E : E + 1], 1.0)

            # ---- PE warm-up (HAM ramp) + exp table-load primer ----
            warm_ps = psS.tile([P, 2 * QBLK], F32, tag="st")
            for _ in range(16):
                nc.tensor.matmul(
                    warm_ps[0:E, 0:256],
                    lhsT=wk_sb[:, 0, :],
                    rhs=wk_sb[:, :, :].rearrange("p c e -> p (c e)")[:, 0:256],
                    start=True,
                    stop=True,
                    skip_group_check=True,
                )
            primer = const.tile([1, 8], BF, tag="primer")
            nc.scalar.activation(
                out=primer,
                in_=ident_sb[0:1, 0:8],
                func=mybir.ActivationFunctionType.Exp,
            )

            # ---- projection of one 512-seq group ----
            def proj_group(g):
                gg, half = g // 2, g % 2
                kvps = psP.tile([P, QBLK], F32, tag="kv", name=f"kvps{g}")
                for c in range(NCH):
                    nc.tensor.matmul(
                        kvps[0:E, :],
                        lhsT=wk_sb[:, c, :],
                        rhs=xk_t[g][:, c, :],
                        start=(c == 0),
                        stop=(c == NCH - 1),
                        tile_position=(0, 0),
                        skip_group_check=True,
                    )
                    nc.tensor.matmul(
                        kvps[E:P, :],
                        lhsT=wv_sb[:, c, :],
                        rhs=xv_t[g][:, c, :],
                        start=(c == 0),
                        stop=(c == NCH - 1),
                        tile_position=(0, E),
                        skip_group_check=True,
                    )
                # one copy moves K^T (rows 0:64) and V^T (rows 64:128) to SBUF
                nc.vector.tensor_copy(
                    kv2_g[gg][:, half * QBLK : (half + 1) * QBLK], kvps
                )
                if g < NQG:
                    qps = psP.tile([P, QBLK], F32, tag="kv", name=f"qps{g}")
                    for c in range(NCH):
                        nc.tensor.matmul(
                            qps[0:E, :],
                            lhsT=wq_sb[:, c, :],
                            rhs=xq_t[g][:, c, :],
                            start=(c == 0),
                            stop=(c == NCH - 1),
                            tile_position=(0, 0),
                            skip_group_check=True,
                        )
                    nc.vector.tensor_copy(qt_b[g], qps[0:E, :])

            # ---- V^T -> V[keys, e] for a 2-group batch (8 chunks) ----
            def vswap_batch(gg):
                vpre = work.tile([P, 2 * NCH, E], BF, tag="vpre", name=f"vpre{gg}")
                kv3 = kv2_g[gg].rearrange("p (c k) -> p c k", c=2 * NCH)
                for bi in range(2):
                    for bj in range(4):
                        nc.gpsimd.dma_start(
                            out=vpre[
                                32 * bj : 32 * bj + 32, :, 32 * bi : 32 * bi + 32
                            ],
                            in_=kv3[
                                E + 32 * bi : E + 32 * bi + 32,
                                :,
                                32 * bj : 32 * bj + 32,
                            ],
                        )
                for bj in range(4):
                    nc.vector.transpose(
                        vp2_g[gg][32 * bj : 32 * bj + 32, :, 0:E],
                        vpre[32 * bj : 32 * bj + 32, :, :],
                    )

            # ---- attention ----
            attn_state = {}

            def attn_begin(ib):
                attn_state[ib] = {
                    "zps": psZ.tile(
                        [E + 1, QBLK], F32, tag="zt", name=f"zps{ib}"
                    ),
                    "prev": None,
                }

            def emit_pv(ib, pt, jp):
                jmax = SLOT_J[ib]
                zps = attn_state[ib]["zps"]
                for h in range(2):
                    j = 2 * jp + h
                    nc.tensor.matmul(
                        zps,
                        lhsT=vp2_g[j // 8][:, j % 8, :],
                        rhs=pt[:, h * QBLK : (h + 1) * QBLK],
                        start=(j == 0),
                        stop=(j == jmax - 1),
                        skip_group_check=True,
                    )

            def attn_pairs(ib, jps):
                jmax = SLOT_J[ib]
                st = attn_state[ib]
                for jp in jps:
                    sps = psS.tile([P, 2 * QBLK], F32, tag="st", name=f"sps{ib}_{jp}")
                    for h in range(2):
                        j = 2 * jp + h
                        kcol = (j % 8) * P
                        kvt = kv2_g[j // 8]
                        # two col-tiles over key halves, shared stationary rows
                        nc.tensor.matmul(
                            sps[0:E, h * QBLK : (h + 1) * QBLK],
                            lhsT=kvt[0:E, kcol : kcol + E],
                            rhs=qt_b[ib],
                            start=True,
                            stop=True,
                            tile_position=(0, 0),
                            skip_group_check=True,
                        )
                        nc.tensor.matmul(
                            sps[E:P, h * QBLK : (h + 1) * QBLK],
                            lhsT=kvt[0:E, kcol + E : kcol + P],
                            rhs=qt_b[ib],
                            start=True,
                            stop=True,
                            tile_position=(0, E),
                            skip_group_check=True,
                        )
                    pt = work.tile([P, 2 * QBLK], BF, tag="pt", name=f"pt{ib}_{jp}")
                    nc.scalar.activation(
                        out=pt, in_=sps, func=mybir.ActivationFunctionType.Exp
                    )
                    j0 = 2 * jp
                    if j0 >= jmax - 8:
                        m = j0 - (jmax - 8)
                        nc.vector.tensor_mul(
                            pt, pt, masks_sb[:, m * QBLK : (m + 2) * QBLK]
                        )
                    if st["prev"] is not None:
                        emit_pv(ib, *st["prev"])
                    st["prev"] = (pt, jp)

            def attn_end(ib):
                st = attn_state[ib]
                emit_pv(ib, *st["prev"])
                zps = st["zps"]
                zsb = epi.tile([E + 1, QBLK], F32, tag="zsb", name=f"zsb{ib}")
                nc.vector.tensor_copy(zsb, zps)
                for u in range(QBLK // P):
                    zbp = psS.tile([P, 2 * QBLK], F32, tag="st", name=f"zbp{ib}_{u}")
                    nc.tensor.transpose(
                        zbp[:, 0 : E + 1],
                        zsb[:, u * P : (u + 1) * P],
                        ident_sb[0 : E + 1, 0 : E + 1],
                    )
                    rc = epi.tile([P, 1], F32, tag="rc", name=f"rc{ib}_{u}")
                    nc.vector.reciprocal(rc, zbp[:, E : E + 1])
                    zf = epi.tile([P, E], F32, tag="zf", name=f"zf{ib}_{u}")
                    nc.vector.tensor_scalar_mul(zf, zbp[:, 0:E], rc)
                    row0 = ib * QBLK + u * P
                    nc.sync.dma_start(out=zout[row0 : row0 + P, :], in_=zf)

            # ---- interleaved schedule (program order == issue order hint) ----
            proj_group(0)
            proj_group(1)
            vswap_batch(0)
            attn_begin(0)
            attn_pairs(0, range(0, 4))      # block 8: chunks 0-7 (groups 0-1)
            attn_end(0)
            attn_begin(1)
            attn_pairs(1, range(0, 4))      # block 16: chunks 0-7
            proj_group(2)
            attn_pairs(1, range(4, 5))      # chunks 8-9 (group 2)
            proj_group(3)
            vswap_batch(1)
            attn_pairs(1, range(5, 8))      # chunks 10-15; PV 8-9 needs vswap 1
            attn_end(1)
            attn_begin(2)
            attn_pairs(2, range(0, 8))      # block 24: chunks 0-15
            proj_group(4)
            attn_pairs(2, range(8, 9))      # chunks 16-17 (group 4)
            proj_group(5)
            vswap_batch(2)
            attn_pairs(2, range(9, 12))     # chunks 18-23; PV 16-17 needs vswap 2
            attn_end(2)
            attn_begin(3)
            attn_pairs(3, range(0, 12))     # block 32: chunks 0-23
            proj_group(6)
            attn_pairs(3, range(12, 13))    # chunks 24-25 (group 6)
            proj_group(7)
            vswap_batch(3)
            attn_pairs(3, range(13, 16))    # chunks 26-31; PV 24-25 needs vswap 3
            attn_end(3)

    nc.compile()
    return nc


def _get_nc():
    if "nc" not in _CACHE:
        _CACHE["nc"] = _build()
    return _CACHE["nc"]


def _ensure_ntff_hook():
    """Install antenv.axon_hooks + NTFF profile hook if the image lacks it."""
    import types

    try:
        from antenv import axon_hooks  # noqa: F401

        return
    except ImportError:
        pass
    import antenv
    from concourse import bass_utils as _bu

    mod = types.ModuleType("antenv.axon_hooks")
    _state = {}
    mod.set_axon_ntff_profile_hook = lambda h: _state.__setitem__("h", h)
    mod.get_axon_ntff_profile_hook = lambda: _state.get("h")
    sys.modules["antenv.axon_hooks"] = mod
    antenv.axon_hooks = mod
    sys.path.insert(0, "/root/.axon_site/trn_agent_boot")
    from trn_boot import _ntff_profile_via_ctypes

    mod.set_axon_ntff_profile_hook(
        _ntff_profile_via_ctypes("/opt/axon/libaxon_pjrt.so")
    )
    _bu.upload_artifacts = lambda tmpdir: f"local://{tmpdir}"


def _make_masks(h):
    kl = np.arange(P)[:, None]
    ql = np.arange(QBLK)[None, :]
    diag = [(kl <= ql - P * t).astype(np.float32) for t in range(4)]
    ones = np.ones((P, QBLK), np.float32)
    zero = np.zeros((P, QBLK), np.float32)
    tiles = diag + [zero] * 4 if h == 0 else [ones] * 4 + diag
    return np.concatenate(tiles, axis=1).astype(BF16)


def kernel(key_inputs, value_inputs, query_inputs, Wq, Wk, Wv):
    global LAST_RESULT
    import os

    key_inputs = np.asarray(key_inputs, dtype=np.float32)
    value_inputs = np.asarray(value_inputs, dtype=np.float32)
    query_inputs = np.asarray(query_inputs, dtype=np.float32)
    wq_b = (np.asarray(Wq, dtype=np.float32) * 0.125).astype(BF16)
    wk_b = np.asarray(Wk, dtype=np.float32).astype(BF16)
    wv_b = np.asarray(Wv, dtype=np.float32).astype(BF16)
    masks_np = [_make_masks(0), _make_masks(1)]
    ident_np = np.eye(P, dtype=np.float32)

    in_maps = []
    for c in range(8):
        b, h = c // 2, c % 2
        xq_c = np.concatenate(
            [query_inputs[b, q0 : q0 + QBLK] for q0 in QSTARTS[h]], axis=0
        )
        xk_c = key_inputs[b]
        xv_c = value_inputs[b]
        in_maps.append(
            {
                "xqt": np.ascontiguousarray(xq_c.T).astype(BF16),
                "xkt": np.ascontiguousarray(xk_c.T).astype(BF16),
                "xvt": np.ascontiguousarray(xv_c.T).astype(BF16),
                "wq": wq_b,
                "wk": wk_b,
                "wv": wv_b,
                "masks": masks_np[h],
                "ident": ident_np,
            }
        )

    nc = _get_nc()
    trace = bool(int(os.environ.get("KERNEL_TRACE", "0")))
    if trace:
        _ensure_ntff_hook()
    res = run_bass_kernel_spmd(
        nc,
        in_maps,
        core_ids=list(range(8)),
        trace=trace,
        tmpdir=os.environ.get("KERNEL_TRACE_DIR") or None,
    )
    LAST_RESULT = res

    out = np.empty((B, S, E), dtype=np.float32)
    for c in range(8):
        b, h = c // 2, c % 2
        z = np.asarray(res.results[c]["z"], dtype=np.float32)
        for ib, q0 in enumerate(QSTARTS[h]):
            out[b, q0 : q0 + QBLK] = z[ib * QBLK : (ib + 1) * QBLK]
    return out


# revision 12
# speedup vs baseline: 1.0122x; 1.0122x over previous
"""Causal attention head (B=4, S=4096, D=512, E=64) on 8 TRN2 NeuronCores.

Sharding: per batch b, core pair (2b, 2b+1); each core owns 2048 queries
(zig-zag slots) and projects K/V for the full sequence.

v3 structure:
 - Per-512-seq-group input tiles, DMA issued in consumption order; compute
   chases the DMA stream (projections + attention interleaved in program
   order).
 - K and V^T projections as col-tiled concurrent matmul pairs
   (wk -> psum[0:64], wv -> psum[64:128]); one DVE copy moves both to SBUF
   (kv2 tiles hold 2 seq groups each).
 - Scores are col-tiled concurrent pairs over key-halves of one chunk
   (keys 0:64 -> psum[0:64], keys 64:128 -> psum[64:128]); both tiles share
   the same K^T stationary rows and Q^T moving operand from partitions 0:64,
   so no partition duplication is needed anywhere.
 - V^T -> V[keys, e] via 32x32 block-swap SBUF DMAs (gpsimd queue, batched
   per 2 groups) + DVE StreamTranspose.
 - PE warm-up matmuls + exp table-load primer at t=0.
 - Flash-style attention: exp on ScalarE over [128, 1024] chunk pairs, PV
   with a ones-column appended to V so the softmax denominator falls out of
   the same matmul. Uniform per-slot key-chunk counts {8,16,24,32}; zig-zag
   query slots; diagonal/zero masks multiply exp output.
All matmul inputs bf16 (pre-cast on host). Output f32.
"""

import sys

sys.path.insert(0, "/opt/trn_rl_repo")

import numpy as np
import ml_dtypes

from concourse import bacc, mybir
from concourse import tile
from concourse.bass_utils import run_bass_kernel_spmd

BF16 = ml_dtypes.bfloat16
F32 = mybir.dt.float32
BF = mybir.dt.bfloat16

B, S, D, E = 4, 4096, 512, 64
P = 128
NQ = 2048          # queries per core
QBLK = 512         # query block
NCH = D // P       # 4 contraction chunks for projections
NG = S // QBLK     # 8 seq groups of 512
NQG = NQ // QBLK   # 4 query groups
QSTARTS = {0: [0, 1024, 2048, 3072], 1: [512, 1536, 2560, 3584]}
SLOT_J = [8, 16, 24, 32]  # uniform per-slot key-chunk counts (all cores)

_CACHE = {}
LAST_RESULT = None


def _build():
    nc = bacc.Bacc(
        "TRN2",
        target_bir_lowering=False,
        debug=False,
        enable_asserts=True,
        num_devices=8,
    )

    xqt_d = nc.declare_dram_parameter("xqt", [D, NQ], BF, isOutput=False)
    xkt_d = nc.declare_dram_parameter("xkt", [D, S], BF, isOutput=False)
    xvt_d = nc.declare_dram_parameter("xvt", [D, S], BF, isOutput=False)
    wq = nc.declare_dram_parameter("wq", [D, E], BF, isOutput=False)  # pre-scaled 1/8
    wk = nc.declare_dram_parameter("wk", [D, E], BF, isOutput=False)
    wv = nc.declare_dram_parameter("wv", [D, E], BF, isOutput=False)
    masks = nc.declare_dram_parameter("masks", [P, 8 * QBLK], BF, isOutput=False)
    ident = nc.declare_dram_parameter("ident", [P, P], F32, isOutput=False)
    zout = nc.declare_dram_parameter("z", [NQ, E], F32, isOutput=True)

    with tile.TileContext(nc) as tc:
        with (
            tc.tile_pool(name="const", bufs=1) as const,
            tc.tile_pool(name="xin", bufs=1) as xin,
            tc.tile_pool(name="proj", bufs=1) as proj,
            tc.tile_pool(name="work", bufs=4) as work,
            tc.tile_pool(name="epi", bufs=2) as epi,
            tc.tile_pool(name="psP", bufs=2, space="PSUM") as psP,
            tc.tile_pool(name="psS", bufs=2, space="PSUM") as psS,
            tc.tile_pool(name="psZ", bufs=2, space="PSUM") as psZ,
        ):
            # ---- constants (DMA first: tiny) ----
            ident_sb = const.tile([P, P], F32, tag="ident")
            nc.sync.dma_start(out=ident_sb[:, :], in_=ident[:, :])
            wq_sb = const.tile([P, NCH, E], BF, tag="wq")
            wk_sb = const.tile([P, NCH, E], BF, tag="wk")
            wv_sb = const.tile([P, NCH, E], BF, tag="wv")
            for w_dram, w_sb in ((wk, wk_sb), (wv, wv_sb), (wq, wq_sb)):
                nc.sync.dma_start(
                    out=w_sb[:, :, :],
                    in_=w_dram.rearrange("(c p) e -> p c e", p=P),
                )

            # ---- input tiles (per 512-seq group, no rotation) ----
            xk_t = [
                xin.tile([P, NCH, QBLK], BF, tag=f"xk{g}", name=f"xk_t{g}")
                for g in range(NG)
            ]
            xv_t = [
                xin.tile([P, NCH, QBLK], BF, tag=f"xv{g}", name=f"xv_t{g}")
                for g in range(NG)
            ]
            xq_t = [
                xin.tile([P, NCH, QBLK], BF, tag=f"xq{b}", name=f"xq_t{b}")
                for b in range(NQG)
            ]
            masks_sb = const.tile([P, 8 * QBLK], BF, tag="masks")

            def load_seg(dram, t, g):
                nc.sync.dma_start(
                    out=t[:, :, :],
                    in_=dram[:, g * QBLK : (g + 1) * QBLK].rearrange(
                        "(c p) r -> p c r", p=P
                    ),
                )

            # DMA issue order == desired arrival order (sync HW queue).
            load_seg(xkt_d, xk_t[0], 0)
            load_seg(xvt_d, xv_t[0], 0)
            load_seg(xqt_d, xq_t[0], 0)
            load_seg(xkt_d, xk_t[1], 1)
            load_seg(xvt_d, xv_t[1], 1)
            nc.sync.dma_start(out=masks_sb[:, :], in_=masks[:, :])
            load_seg(xqt_d, xq_t[1], 1)
            for g in range(2, NG):
                load_seg(xkt_d, xk_t[g], g)
                load_seg(xvt_d, xv_t[g], g)
                if g < NQG:
                    load_seg(xqt_d, xq_t[g], g)

            # ---- projected-data tiles ----
            # kv2_g[gg] holds 2 seq groups: K^T in rows 0:64, V^T in 64:128
            kv2_g = [
                proj.tile([P, 2 * QBLK], BF, tag=f"kv{gg}", name=f"kv2_g{gg}")
                for gg in range(NG // 2)
            ]
            qt_b = [
                proj.tile([E, QBLK], BF, tag=f"qt{b}", name=f"qt_b{b}")
                for b in range(NQG)
            ]
            # vp2_g[gg]: V[keys, e] (+ones col) for 8 chunks (2 seq groups)
            vp2_g = [
                proj.tile([P, 2 * NCH, E + 1], BF, tag=f"vp{gg}", name=f"vp2_g{gg}")
                for gg in range(NG // 2)
            ]
            for gg in range(NG // 2):
                nc.gpsimd.memset(vp2_g[gg][:, :, E : E + 1], 1.0)

            # ---- PE warm-up (HAM ramp) + exp table-load primer ----
            warm_ps = psS.tile([P, 2 * QBLK], F32, tag="st")
            for _ in range(16):
                nc.tensor.matmul(
                    warm_ps[0:E, 0:256],
                    lhsT=wk_sb[:, 0, :],
                    rhs=wk_sb[:, :, :].rearrange("p c e -> p (c e)")[:, 0:256],
                    start=True,
                    stop=True,
                    skip_group_check=True,
                )
            primer = const.tile([1, 8], BF, tag="primer")
            nc.scalar.activation(
                out=primer,
                in_=ident_sb[0:1, 0:8],
                func=mybir.ActivationFunctionType.Exp,
            )

            # ---- projection, emitted in per-chunk units so it can interleave
            # with attention iterations (keeps PE duty high for HAM) ----
            proj_ps = {}

            def kv_unit(g, c):
                """One contraction step of the (K,V) col-tiled pair for group g."""
                if c == 0:
                    proj_ps[g] = psP.tile([P, QBLK], F32, tag="kv", name=f"kvps{g}")
                kvps = proj_ps[g]
                nc.tensor.matmul(
                    kvps[0:E, :],
                    lhsT=wk_sb[:, c, :],
                    rhs=xk_t[g][:, c, :],
                    start=(c == 0),
                    stop=(c == NCH - 1),
                    tile_position=(0, 0),
                    skip_group_check=True,
                )
                nc.tensor.matmul(
                    kvps[E:P, :],
                    lhsT=wv_sb[:, c, :],
                    rhs=xv_t[g][:, c, :],
                    start=(c == 0),
                    stop=(c == NCH - 1),
                    tile_position=(0, E),
                    skip_group_check=True,
                )
                if c == NCH - 1:
                    gg, half = g // 2, g % 2
                    nc.vector.tensor_copy(
                        kv2_g[gg][:, half * QBLK : (half + 1) * QBLK], kvps
                    )

            def q_unit(b, c):
                """One contraction step of the Q projection for block b."""
                if c == 0:
                    proj_ps[f"q{b}"] = psP.tile(
                        [P, QBLK], F32, tag="kv", name=f"qps{b}"
                    )
                qps = proj_ps[f"q{b}"]
                nc.tensor.matmul(
                    qps[0:E, :],
                    lhsT=wq_sb[:, c, :],
                    rhs=xq_t[b][:, c, :],
                    start=(c == 0),
                    stop=(c == NCH - 1),
                    tile_position=(0, 0),
                    skip_group_check=True,
                )
                if c == NCH - 1:
                    nc.vector.tensor_copy(qt_b[b], qps[0:E, :])

            def proj_group(g):
                for c in range(NCH):
                    kv_unit(g, c)
                if g < 2:
                    for c in range(NCH):
                        q_unit(g, c)

            # ---- V^T -> V[keys, e] for a 2-group batch (8 chunks) ----
            def vswap_batch(gg):
                vpre = work.tile([P, 2 * NCH, E], BF, tag="vpre", name=f"vpre{gg}")
                kv3 = kv2_g[gg].rearrange("p (c k) -> p c k", c=2 * NCH)
                for bi in range(2):
                    for bj in range(4):
                        nc.gpsimd.dma_start(
                            out=vpre[
                                32 * bj : 32 * bj + 32, :, 32 * bi : 32 * bi + 32
                            ],
                            in_=kv3[
                                E + 32 * bi : E + 32 * bi + 32,
                                :,
                                32 * bj : 32 * bj + 32,
                            ],
                        )
                for bj in range(4):
                    nc.vector.transpose(
                        vp2_g[gg][32 * bj : 32 * bj + 32, :, 0:E],
                        vpre[32 * bj : 32 * bj + 32, :, :],
                    )

            # ---- attention ----
            attn_state = {}

            def attn_begin(ib):
                attn_state[ib] = {
                    "zps": psZ.tile(
                        [E + 1, QBLK], F32, tag="zt", name=f"zps{ib}"
                    ),
                    "pend": [],
                }

            def emit_pv(ib, pt, jp):
                jmax = SLOT_J[ib]
                zps = attn_state[ib]["zps"]
                for h in range(2):
                    j = 2 * jp + h
                    nc.tensor.matmul(
                        zps,
                        lhsT=vp2_g[j // 8][:, j % 8, :],
                        rhs=pt[:, h * QBLK : (h + 1) * QBLK],
                        start=(j == 0),
                        stop=(j == jmax - 1),
                        skip_group_check=True,
                    )

            def attn_pairs(ib, jps, fillers=None):
                jmax = SLOT_J[ib]
                st = attn_state[ib]
                for jp in jps:
                    sps = psS.tile([P, 2 * QBLK], F32, tag="st", name=f"sps{ib}_{jp}")
                    for h in range(2):
                        j = 2 * jp + h
                        kcol = (j % 8) * P
                        kvt = kv2_g[j // 8]
                        # two col-tiles over key halves, shared stationary rows
                        nc.tensor.matmul(
                            sps[0:E, h * QBLK : (h + 1) * QBLK],
                            lhsT=kvt[0:E, kcol : kcol + E],
                            rhs=qt_b[ib],
                            start=True,
                            stop=True,
                            tile_position=(0, 0),
                            skip_group_check=True,
                        )
                        nc.tensor.matmul(
                            sps[E:P, h * QBLK : (h + 1) * QBLK],
                            lhsT=kvt[0:E, kcol + E : kcol + P],
                            rhs=qt_b[ib],
                            start=True,
                            stop=True,
                            tile_position=(0, E),
                            skip_group_check=True,
                        )
                    pt = work.tile([P, 2 * QBLK], BF, tag="pt", name=f"pt{ib}_{jp}")
                    nc.scalar.activation(
                        out=pt, in_=sps, func=mybir.ActivationFunctionType.Exp
                    )
                    j0 = 2 * jp
                    if j0 >= jmax - 8:
                        m = j0 - (jmax - 8)
                        nc.vector.tensor_mul(
                            pt, pt, masks_sb[:, m * QBLK : (m + 2) * QBLK]
                        )
                    for f in (fillers or {}).get(jp, []):
                        f()
                    st["pend"].append((pt, jp))
                    if len(st["pend"]) > 2:
                        emit_pv(ib, *st["pend"].pop(0))

            def attn_end(ib):
                st = attn_state[ib]
                for item in st["pend"]:
                    emit_pv(ib, *item)
                st["pend"] = []
                zps = st["zps"]
                zsb = epi.tile([E + 1, QBLK], F32, tag="zsb", name=f"zsb{ib}")
                nc.vector.tensor_copy(zsb, zps)
                for u in range(QBLK // P):
                    zbp = psS.tile([P, 2 * QBLK], F32, tag="st", name=f"zbp{ib}_{u}")
                    nc.tensor.transpose(
                        zbp[:, 0 : E + 1],
                        zsb[:, u * P : (u + 1) * P],
                        ident_sb[0 : E + 1, 0 : E + 1],
                    )
                    rc = epi.tile([P, 1], F32, tag="rc", name=f"rc{ib}_{u}")
                    nc.vector.reciprocal(rc, zbp[:, E : E + 1])
                    zf = epi.tile([P, E], F32, tag="zf", name=f"zf{ib}_{u}")
                    nc.vector.tensor_scalar_mul(zf, zbp[:, 0:E], rc)
                    row0 = ib * QBLK + u * P
                    nc.sync.dma_start(out=zout[row0 : row0 + P, :], in_=zf)

            # ---- interleaved schedule: projection units ride inside the
            # attention iteration stream so PE duty stays above the HAM
            # re-throttle threshold ----
            def KV(g, c):
                return lambda: kv_unit(g, c)

            def Q(b, c):
                return lambda: q_unit(b, c)

            proj_group(0)
            proj_group(1)
            vswap_batch(0)
            attn_begin(0)
            attn_pairs(0, range(0, 4))      # block 8: chunks 0-7 (groups 0-1)
            attn_end(0)
            attn_begin(1)                   # block 16
            attn_pairs(
                1,
                range(0, 6),
                fillers={
                    1: [KV(2, 0)],
                    2: [KV(2, 1)],
                    3: [KV(2, 2), KV(2, 3)],
                    4: [KV(3, 0), KV(3, 1)],
                    5: [KV(3, 2), KV(3, 3)],
                },
            )
            vswap_batch(1)
            attn_pairs(
                1,
                range(6, 8),
                fillers={6: [Q(2, 0), Q(2, 1)], 7: [Q(2, 2), Q(2, 3)]},
            )
            attn_end(1)
            attn_begin(2)                   # block 24
            attn_pairs(
                2,
                range(0, 8),
                fillers={
                    1: [KV(4, 0)],
                    2: [KV(4, 1)],
                    3: [KV(4, 2), KV(4, 3)],
                    4: [Q(3, 0), Q(3, 1)],
                    5: [Q(3, 2), Q(3, 3)],
                    6: [KV(5, 0), KV(5, 1)],
                    7: [KV(5, 2), KV(5, 3)],
                },
            )
            vswap_batch(2)
            attn_pairs(2, range(8, 12))
            attn_end(2)
            attn_begin(3)                   # block 32
            attn_pairs(
                3,
                range(0, 7),
                fillers={
                    1: [KV(6, 0)],
                    2: [KV(6, 1)],
                    3: [KV(6, 2), KV(6, 3)],
                    5: [KV(7, 0), KV(7, 1)],
                    6: [KV(7, 2), KV(7, 3)],
                },
            )
            vswap_batch(3)
            attn_pairs(3, range(7, 16))
            attn_end(3)

    nc.compile()
    return nc


def _get_nc():
    if "nc" not in _CACHE:
        _CACHE["nc"] = _build()
    return _CACHE["nc"]


def _ensure_ntff_hook():
    """Install antenv.axon_hooks + NTFF profile hook if the image lacks it."""
    import types

    try:
        from antenv import axon_hooks  # noqa: F401

        return
    except ImportError:
        pass
    import antenv
    from concourse import bass_utils as _bu

    mod = types.ModuleType("antenv.axon_hooks")
    _state = {}
    mod.set_axon_ntff_profile_hook = lambda h: _state.__setitem__("h", h)
    mod.get_axon_ntff_profile_hook = lambda: _state.get("h")
    sys.modules["antenv.axon_hooks"] = mod
    antenv.axon_hooks = mod
    sys.path.insert(0, "/root/.axon_site/trn_agent_boot")
    from trn_boot import _ntff_profile_via_ctypes

    mod.set_axon_ntff_profile_hook(
        _ntff_profile_via_ctypes("/opt/axon/libaxon_pjrt.so")
    )
    _bu.upload_artifacts = lambda tmpdir: f"local://{tmpdir}"


def _make_masks(h):
    kl = np.arange(P)[:, None]
    ql = np.arange(QBLK)[None, :]
    diag = [(kl <= ql - P * t).astype(np.float32) for t in range(4)]
    ones = np.ones((P, QBLK), np.float32)
    zero = np.zeros((P, QBLK), np.float32)
    tiles = diag + [zero] * 4 if h == 0 else [ones] * 4 + diag
    return np.concatenate(tiles, axis=1).astype(BF16)


def kernel(key_inputs, value_inputs, query_inputs, Wq, Wk, Wv):
    global LAST_RESULT
    import os

    key_inputs = np.asarray(key_inputs, dtype=np.float32)
    value_inputs = np.asarray(value_inputs, dtype=np.float32)
    query_inputs = np.asarray(query_inputs, dtype=np.float32)
    wq_b = (np.asarray(Wq, dtype=np.float32) * 0.125).astype(BF16)
    wk_b = np.asarray(Wk, dtype=np.float32).astype(BF16)
    wv_b = np.asarray(Wv, dtype=np.float32).astype(BF16)
    masks_np = [_make_masks(0), _make_masks(1)]
    ident_np = np.eye(P, dtype=np.float32)

    in_maps = []
    for c in range(8):
        b, h = c // 2, c % 2
        xq_c = np.concatenate(
            [query_inputs[b, q0 : q0 + QBLK] for q0 in QSTARTS[h]], axis=0
        )
        xk_c = key_inputs[b]
        xv_c = value_inputs[b]
        in_maps.append(
            {
                "xqt": np.ascontiguousarray(xq_c.T).astype(BF16),
                "xkt": np.ascontiguousarray(xk_c.T).astype(BF16),
                "xvt": np.ascontiguousarray(xv_c.T).astype(BF16),
                "wq": wq_b,
                "wk": wk_b,
                "wv": wv_b,
                "masks": masks_np[h],
                "ident": ident_np,
            }
        )

    nc = _get_nc()
    trace = bool(int(os.environ.get("KERNEL_TRACE", "0")))
    if trace:
        _ensure_ntff_hook()
    res = run_bass_kernel_spmd(
        nc,
        in_maps,
        core_ids=list(range(8)),
        trace=trace,
        tmpdir=os.environ.get("KERNEL_TRACE_DIR") or None,
    )
    LAST_RESULT = res

    out = np.empty((B, S, E), dtype=np.float32)
    for c in range(8):
        b, h = c // 2, c % 2
        z = np.asarray(res.results[c]["z"], dtype=np.float32)
        for ib, q0 in enumerate(QSTARTS[h]):
            out[b, q0 : q0 + QBLK] = z[ib * QBLK : (ib + 1) * QBLK]
    return out


# revision 18
# speedup vs baseline: 1.1749x; 1.1608x over previous
"""Causal attention head (B=4, S=4096, D=512, E=64) on 8 TRN2 NeuronCores.

Sharding: per batch b, core pair (2b, 2b+1); each core owns 2048 queries
(zig-zag slots) and projects K/V for the full sequence.

v3 structure:
 - Per-512-seq-group input tiles, DMA issued in consumption order; compute
   chases the DMA stream (projections + attention interleaved in program
   order).
 - K and V^T projections as col-tiled concurrent matmul pairs
   (wk -> psum[0:64], wv -> psum[64:128]); one DVE copy moves both to SBUF
   (kv2 tiles hold 2 seq groups each).
 - Scores are col-tiled concurrent pairs over key-halves of one chunk
   (keys 0:64 -> psum[0:64], keys 64:128 -> psum[64:128]); both tiles share
   the same K^T stationary rows and Q^T moving operand from partitions 0:64,
   so no partition duplication is needed anywhere.
 - V^T -> V[keys, e] via 32x32 block-swap SBUF DMAs (gpsimd queue, batched
   per 2 groups) + DVE StreamTranspose.
 - PE warm-up matmuls + exp table-load primer at t=0.
 - Flash-style attention: exp on ScalarE over [128, 1024] chunk pairs, PV
   with a ones-column appended to V so the softmax denominator falls out of
   the same matmul. Uniform per-slot key-chunk counts {8,16,24,32}; zig-zag
   query slots; diagonal/zero masks multiply exp output.
All matmul inputs bf16 (pre-cast on host). Output f32.
"""

import sys

sys.path.insert(0, "/opt/trn_rl_repo")

import numpy as np
import ml_dtypes

from concourse import bacc, mybir
from concourse import tile
from concourse.bass_utils import run_bass_kernel_spmd

BF16 = ml_dtypes.bfloat16
F32 = mybir.dt.float32
BF = mybir.dt.bfloat16

B, S, D, E = 4, 4096, 512, 64
P = 128
NQ = 2048          # queries per core
QBLK = 512         # query block
NCH = D // P       # 4 contraction chunks for projections
NG = S // QBLK     # 8 seq groups of 512
NQG = NQ // QBLK   # 4 query groups
QSTARTS = {0: [0, 1024, 2048, 3072], 1: [512, 1536, 2560, 3584]}
SLOT_J = [8, 16, 24, 32]  # uniform per-slot key-chunk counts (all cores)

_CACHE = {}
LAST_RESULT = None


def _build():
    nc = bacc.Bacc(
        "TRN2",
        target_bir_lowering=False,
        debug=False,
        enable_asserts=True,
        num_devices=8,
    )

    xqt_d = nc.declare_dram_parameter("xqt", [D, NQ], BF, isOutput=False)
    xkt_d = nc.declare_dram_parameter("xkt", [D, S], BF, isOutput=False)
    xvt_d = nc.declare_dram_parameter("xvt", [D, S], BF, isOutput=False)
    wq = nc.declare_dram_parameter("wq", [D, E], BF, isOutput=False)  # pre-scaled 1/8
    wk = nc.declare_dram_parameter("wk", [D, E], BF, isOutput=False)
    wv = nc.declare_dram_parameter("wv", [D, E], BF, isOutput=False)
    masks = nc.declare_dram_parameter("masks", [P, 8 * QBLK], BF, isOutput=False)
    ident = nc.declare_dram_parameter("ident", [P, P], F32, isOutput=False)
    zout = nc.declare_dram_parameter("z", [NQ, E], F32, isOutput=True)

    with tile.TileContext(nc) as tc:
        with (
            tc.tile_pool(name="const", bufs=1) as const,
            tc.tile_pool(name="xin", bufs=1) as xin,
            tc.tile_pool(name="proj", bufs=1) as proj,
            tc.tile_pool(name="work", bufs=4) as work,
            tc.tile_pool(name="epi", bufs=2) as epi,
            tc.tile_pool(name="psP", bufs=2, space="PSUM") as psP,
            tc.tile_pool(name="psS", bufs=2, space="PSUM") as psS,
            tc.tile_pool(name="psZ", bufs=2, space="PSUM") as psZ,
        ):
            # ---- constants (DMA first: tiny) ----
            ident_sb = const.tile([P, P], F32, tag="ident")
            nc.sync.dma_start(out=ident_sb[:, :], in_=ident[:, :])
            wq_sb = const.tile([P, NCH, E], BF, tag="wq")
            wk_sb = const.tile([P, NCH, E], BF, tag="wk")
            wv_sb = const.tile([P, NCH, E], BF, tag="wv")
            for w_dram, w_sb in ((wk, wk_sb), (wv, wv_sb), (wq, wq_sb)):
                nc.sync.dma_start(
                    out=w_sb[:, :, :],
                    in_=w_dram.rearrange("(c p) e -> p c e", p=P),
                )

            # ---- input tiles (per 512-seq group, no rotation) ----
            xk_t = [
                xin.tile([P, NCH, QBLK], BF, tag=f"xk{g}", name=f"xk_t{g}")
                for g in range(NG)
            ]
            xv_t = [
                xin.tile([P, NCH, QBLK], BF, tag=f"xv{g}", name=f"xv_t{g}")
                for g in range(NG)
            ]
            xq_t = [
                xin.tile([P, NCH, QBLK], BF, tag=f"xq{b}", name=f"xq_t{b}")
                for b in range(NQG)
            ]
            masks_sb = const.tile([P, 8 * QBLK], BF, tag="masks")

            def load_seg(dram, t, g):
                nc.sync.dma_start(
                    out=t[:, :, :],
                    in_=dram[:, g * QBLK : (g + 1) * QBLK].rearrange(
                        "(c p) r -> p c r", p=P
                    ),
                )

            # DMA issue order == desired arrival order (sync HW queue).
            load_seg(xkt_d, xk_t[0], 0)
            load_seg(xvt_d, xv_t[0], 0)
            load_seg(xqt_d, xq_t[0], 0)
            load_seg(xkt_d, xk_t[1], 1)
            load_seg(xvt_d, xv_t[1], 1)
            nc.sync.dma_start(out=masks_sb[:, :], in_=masks[:, :])
            load_seg(xqt_d, xq_t[1], 1)
            for g in range(2, NG):
                load_seg(xkt_d, xk_t[g], g)
                load_seg(xvt_d, xv_t[g], g)
                if g < NQG:
                    load_seg(xqt_d, xq_t[g], g)

            # ---- projected-data tiles ----
            # kv2_g[gg] holds 2 seq groups: K^T in rows 0:64, V^T in 64:128
            kv2_g = [
                proj.tile([P, 2 * QBLK], BF, tag=f"kv{gg}", name=f"kv2_g{gg}")
                for gg in range(NG // 2)
            ]
            # ktb_g[gg]: K^T duplicated into partitions 64:128 (row-tiled scores)
            ktb_g = [
                proj.tile([P, 2 * QBLK], BF, tag=f"kt{gg}", name=f"ktb_g{gg}")
                for gg in range(NG // 2)
            ]
            qt_b = [
                proj.tile([P, QBLK], BF, tag=f"qt{b}", name=f"qt_b{b}")
                for b in range(NQG)
            ]
            # vp2_g[gg]: V[keys, e] (+ones col) for 8 chunks (2 seq groups)
            vp2_g = [
                proj.tile([P, 2 * NCH, E + 1], BF, tag=f"vp{gg}", name=f"vp2_g{gg}")
                for gg in range(NG // 2)
            ]
            for gg in range(NG // 2):
                nc.gpsimd.memset(vp2_g[gg][:, :, E : E + 1], 1.0)

            # ---- PE warm-up (HAM ramp) + exp table-load primer ----
            warm_ps = psS.tile([P, 2 * QBLK], F32, tag="st")
            for _ in range(16):
                nc.tensor.matmul(
                    warm_ps[0:E, 0:256],
                    lhsT=wk_sb[:, 0, :],
                    rhs=wk_sb[:, :, :].rearrange("p c e -> p (c e)")[:, 0:256],
                    start=True,
                    stop=True,
                    skip_group_check=True,
                )
            primer = const.tile([1, 8], BF, tag="primer")
            nc.scalar.activation(
                out=primer,
                in_=ident_sb[0:1, 0:8],
                func=mybir.ActivationFunctionType.Exp,
            )

            # ---- projection, emitted in per-chunk units so it can interleave
            # with attention iterations (keeps PE duty high for HAM) ----
            proj_ps = {}

            def kv_unit(g, c):
                """One contraction step of the (K,V) col-tiled pair for group g."""
                if c == 0:
                    proj_ps[g] = psP.tile([P, QBLK], F32, tag="kv", name=f"kvps{g}")
                kvps = proj_ps[g]
                nc.tensor.matmul(
                    kvps[0:E, :],
                    lhsT=wk_sb[:, c, :],
                    rhs=xk_t[g][:, c, :],
                    start=(c == 0),
                    stop=(c == NCH - 1),
                    tile_position=(0, 0),
                    skip_group_check=True,
                )
                nc.tensor.matmul(
                    kvps[E:P, :],
                    lhsT=wv_sb[:, c, :],
                    rhs=xv_t[g][:, c, :],
                    start=(c == 0),
                    stop=(c == NCH - 1),
                    tile_position=(0, E),
                    skip_group_check=True,
                )
                if c == NCH - 1:
                    gg, half = g // 2, g % 2
                    sl = slice(half * QBLK, (half + 1) * QBLK)
                    nc.vector.tensor_copy(kv2_g[gg][:, sl], kvps)
                    # K^T partition-dup for the row-tiled scores; groups 0/1 on
                    # the scalar queue (needed early), later groups on gpsimd
                    dma_eng = nc.scalar if g < 2 else nc.gpsimd
                    dma_eng.dma_start(
                        out=ktb_g[gg][E:P, sl], in_=kv2_g[gg][0:E, sl]
                    )

            def q_unit(b, c):
                """One contraction step of the Q projection for block b."""
                if c == 0:
                    proj_ps[f"q{b}"] = psP.tile(
                        [P, QBLK], F32, tag="kv", name=f"qps{b}"
                    )
                qps = proj_ps[f"q{b}"]
                nc.tensor.matmul(
                    qps[0:E, :],
                    lhsT=wq_sb[:, c, :],
                    rhs=xq_t[b][:, c, :],
                    start=(c == 0),
                    stop=(c == NCH - 1),
                    tile_position=(0, 0),
                    skip_group_check=True,
                )
                if c == NCH - 1:
                    nc.vector.tensor_copy(qt_b[b][0:E, :], qps[0:E, :])
                    # duplicate Q^T into partitions 64:128 (row-tiled scores);
                    # blocks 0/1 on the scalar queue (needed early), 2/3 gpsimd
                    dma_eng = nc.scalar if b < 2 else nc.gpsimd
                    dma_eng.dma_start(out=qt_b[b][E:P, :], in_=qt_b[b][0:E, :])

            def proj_group(g):
                for c in range(NCH):
                    kv_unit(g, c)
                if g < 2:
                    for c in range(NCH):
                        q_unit(g, c)

            # ---- V^T -> V[keys, e] for a 2-group batch (8 chunks) ----
            def vswap_batch(gg):
                vpre = work.tile([P, 2 * NCH, E], BF, tag="vpre", name=f"vpre{gg}")
                kv3 = kv2_g[gg].rearrange("p (c k) -> p c k", c=2 * NCH)
                for bi in range(2):
                    for bj in range(4):
                        nc.gpsimd.dma_start(
                            out=vpre[
                                32 * bj : 32 * bj + 32, :, 32 * bi : 32 * bi + 32
                            ],
                            in_=kv3[
                                E + 32 * bi : E + 32 * bi + 32,
                                :,
                                32 * bj : 32 * bj + 32,
                            ],
                        )
                for bj in range(4):
                    nc.vector.transpose(
                        vp2_g[gg][32 * bj : 32 * bj + 32, :, 0:E],
                        vpre[32 * bj : 32 * bj + 32, :, :],
                    )

            # ---- attention ----
            attn_state = {}

            def attn_begin(ib):
                attn_state[ib] = {
                    "zps": psZ.tile(
                        [E + 1, QBLK], F32, tag="zt", name=f"zps{ib}"
                    ),
                    "pend": [],
                }

            def emit_pv(ib, pt, jp):
                jmax = SLOT_J[ib]
                zps = attn_state[ib]["zps"]
                for h in range(2):
                    j = 2 * jp + h
                    nc.tensor.matmul(
                        zps,
                        lhsT=vp2_g[j // 8][:, j % 8, :],
                        rhs=pt[:, h * QBLK : (h + 1) * QBLK],
                        start=(j == 0),
                        stop=(j == jmax - 1),
                        skip_group_check=True,
                    )

            def attn_pairs(ib, jps, fillers=None):
                jmax = SLOT_J[ib]
                st = attn_state[ib]
                for jp in jps:
                    sps = psS.tile([P, 2 * QBLK], F32, tag="st", name=f"sps{ib}_{jp}")
                    j0, j1 = 2 * jp, 2 * jp + 1
                    # row-tiled concurrent pair: chunk j0 on array rows 0:64,
                    # chunk j1 on rows 64:128 (reads the partition-dup copies)
                    nc.tensor.matmul(
                        sps[:, 0:QBLK],
                        lhsT=kv2_g[j0 // 8][0:E, (j0 % 8) * P : (j0 % 8 + 1) * P],
                        rhs=qt_b[ib][0:E, :],
                        start=True,
                        stop=True,
                        tile_position=(0, 0),
                        skip_group_check=True,
                    )
                    nc.tensor.matmul(
                        sps[:, QBLK : 2 * QBLK],
                        lhsT=ktb_g[j1 // 8][E:P, (j1 % 8) * P : (j1 % 8 + 1) * P],
                        rhs=qt_b[ib][E:P, :],
                        start=True,
                        stop=True,
                        tile_position=(E, 0),
                        skip_group_check=True,
                    )
                    pt = work.tile([P, 2 * QBLK], BF, tag="pt", name=f"pt{ib}_{jp}")
                    nc.scalar.activation(
                        out=pt, in_=sps, func=mybir.ActivationFunctionType.Exp
                    )
                    j0 = 2 * jp
                    if j0 >= jmax - 8:
                        m = j0 - (jmax - 8)
                        nc.vector.tensor_mul(
                            pt, pt, masks_sb[:, m * QBLK : (m + 2) * QBLK]
                        )
                    for f in (fillers or {}).get(jp, []):
                        f()
                    st["pend"].append((pt, jp))
                    if len(st["pend"]) > 2:
                        emit_pv(ib, *st["pend"].pop(0))

            def attn_end(ib):
                st = attn_state[ib]
                for item in st["pend"]:
                    emit_pv(ib, *item)
                st["pend"] = []
                zps = st["zps"]
                zsb = epi.tile([E + 1, QBLK], F32, tag="zsb", name=f"zsb{ib}")
                nc.vector.tensor_copy(zsb, zps)
                for u in range(QBLK // P):
                    zbp = psS.tile([P, 2 * QBLK], F32, tag="st", name=f"zbp{ib}_{u}")
                    nc.tensor.transpose(
                        zbp[:, 0 : E + 1],
                        zsb[:, u * P : (u + 1) * P],
                        ident_sb[0 : E + 1, 0 : E + 1],
                    )
                    rc = epi.tile([P, 1], F32, tag="rc", name=f"rc{ib}_{u}")
                    nc.vector.reciprocal(rc, zbp[:, E : E + 1])
                    zf = epi.tile([P, E], F32, tag="zf", name=f"zf{ib}_{u}")
                    nc.vector.tensor_scalar_mul(zf, zbp[:, 0:E], rc)
                    row0 = ib * QBLK + u * P
                    nc.sync.dma_start(out=zout[row0 : row0 + P, :], in_=zf)

            # ---- interleaved schedule: projection units ride inside the
            # attention iteration stream so PE duty stays above the HAM
            # re-throttle threshold ----
            def KV(g, c):
                return lambda: kv_unit(g, c)

            def Q(b, c):
                return lambda: q_unit(b, c)

            proj_group(0)
            proj_group(1)
            vswap_batch(0)
            attn_begin(0)
            attn_pairs(0, range(0, 4))      # block 8: chunks 0-7 (groups 0-1)
            attn_end(0)
            attn_begin(1)                   # block 16
            attn_pairs(
                1,
                range(0, 6),
                fillers={
                    1: [KV(2, 0)],
                    2: [KV(2, 1)],
                    3: [KV(2, 2), KV(2, 3)],
                    4: [KV(3, 0), KV(3, 1)],
                    5: [KV(3, 2), KV(3, 3)],
                },
            )
            vswap_batch(1)
            attn_pairs(
                1,
                range(6, 8),
                fillers={6: [Q(2, 0), Q(2, 1)], 7: [Q(2, 2), Q(2, 3)]},
            )
            attn_end(1)
            attn_begin(2)                   # block 24
            attn_pairs(
                2,
                range(0, 8),
                fillers={
                    1: [KV(4, 0)],
                    2: [KV(4, 1)],
                    3: [KV(4, 2), KV(4, 3)],
                    4: [Q(3, 0), Q(3, 1)],
                    5: [Q(3, 2), Q(3, 3)],
                    6: [KV(5, 0), KV(5, 1)],
                    7: [KV(5, 2), KV(5, 3)],
                },
            )
            vswap_batch(2)
            attn_pairs(2, range(8, 12))
            attn_end(2)
            attn_begin(3)                   # block 32
            attn_pairs(
                3,
                range(0, 7),
                fillers={
                    1: [KV(6, 0)],
                    2: [KV(6, 1)],
                    3: [KV(6, 2), KV(6, 3)],
                    5: [KV(7, 0), KV(7, 1)],
                    6: [KV(7, 2), KV(7, 3)],
                },
            )
            vswap_batch(3)
            attn_pairs(3, range(7, 16))
            attn_end(3)

    nc.compile()
    return nc


def _get_nc():
    if "nc" not in _CACHE:
        _CACHE["nc"] = _build()
    return _CACHE["nc"]


def _ensure_ntff_hook():
    """Install antenv.axon_hooks + NTFF profile hook if the image lacks it."""
    import types

    try:
        from antenv import axon_hooks  # noqa: F401

        return
    except ImportError:
        pass
    import antenv
    from concourse import bass_utils as _bu

    mod = types.ModuleType("antenv.axon_hooks")
    _state = {}
    mod.set_axon_ntff_profile_hook = lambda h: _state.__setitem__("h", h)
    mod.get_axon_ntff_profile_hook = lambda: _state.get("h")
    sys.modules["antenv.axon_hooks"] = mod
    antenv.axon_hooks = mod
    sys.path.insert(0, "/root/.axon_site/trn_agent_boot")
    from trn_boot import _ntff_profile_via_ctypes

    mod.set_axon_ntff_profile_hook(
        _ntff_profile_via_ctypes("/opt/axon/libaxon_pjrt.so")
    )
    _bu.upload_artifacts = lambda tmpdir: f"local://{tmpdir}"


def _make_masks(h):
    kl = np.arange(P)[:, None]
    ql = np.arange(QBLK)[None, :]
    diag = [(kl <= ql - P * t).astype(np.float32) for t in range(4)]
    ones = np.ones((P, QBLK), np.float32)
    zero = np.zeros((P, QBLK), np.float32)
    tiles = diag + [zero] * 4 if h == 0 else [ones] * 4 + diag
    return np.concatenate(tiles, axis=1).astype(BF16)


def kernel(key_inputs, value_inputs, query_inputs, Wq, Wk, Wv):
    global LAST_RESULT
    import os

    key_inputs = np.asarray(key_inputs, dtype=np.float32)
    value_inputs = np.asarray(value_inputs, dtype=np.float32)
    query_inputs = np.asarray(query_inputs, dtype=np.float32)
    wq_b = (np.asarray(Wq, dtype=np.float32) * 0.125).astype(BF16)
    wk_b = np.asarray(Wk, dtype=np.float32).astype(BF16)
    wv_b = np.asarray(Wv, dtype=np.float32).astype(BF16)
    masks_np = [_make_masks(0), _make_masks(1)]
    ident_np = np.eye(P, dtype=np.float32)

    in_maps = []
    for c in range(8):
        b, h = c // 2, c % 2
        xq_c = np.concatenate(
            [query_inputs[b, q0 : q0 + QBLK] for q0 in QSTARTS[h]], axis=0
        )
        xk_c = key_inputs[b]
        xv_c = value_inputs[b]
        in_maps.append(
            {
                "xqt": np.ascontiguousarray(xq_c.T).astype(BF16),
                "xkt": np.ascontiguousarray(xk_c.T).astype(BF16),
                "xvt": np.ascontiguousarray(xv_c.T).astype(BF16),
                "wq": wq_b,
                "wk": wk_b,
                "wv": wv_b,
                "masks": masks_np[h],
                "ident": ident_np,
            }
        )

    nc = _get_nc()
    trace = bool(int(os.environ.get("KERNEL_TRACE", "0")))
    if trace:
        _ensure_ntff_hook()
    res = run_bass_kernel_spmd(
        nc,
        in_maps,
        core_ids=list(range(8)),
        trace=trace,
        tmpdir=os.environ.get("KERNEL_TRACE_DIR") or None,
    )
    LAST_RESULT = res

    out = np.empty((B, S, E), dtype=np.float32)
    for c in range(8):
        b, h = c // 2, c % 2
        z = np.asarray(res.results[c]["z"], dtype=np.float32)
        for ib, q0 in enumerate(QSTARTS[h]):
            out[b, q0 : q0 + QBLK] = z[ib * QBLK : (ib + 1) * QBLK]
    return out
